# revision 12
# baseline (speedup 1.0000x reference)
"""Trainium2 Bass kernel for a Conformer-style convolution module.

Computation (per reference):
  y   = GLU(x @ w1.T + b1)                       # pointwise conv1 + GLU
  seq = concat(cache, y-flattened) ; right-pad   # (C, 7 + B*T + 7)
  c   = depthwise_conv_K15(seq) + db             # (C, B*T)
  z   = relu(LN_over_channels(c) * gamma + beta)
  out = z @ w2.T + b2

Sharding: batch/time-parallel over the flattened B*T=16384 frame axis,
2048 frames per core on 8 cores.  The depthwise conv's +-7 frame halo is
supplied per-core by tiny host-side GLU computations (or the cache /
zero-padding at the global edges), so cores need no collectives.

On-chip layout is channel-major (C on partitions, time on the free dim).
 - GEMM1/GEMM2 run on the PE in bf16 with fp32 PSUM accumulation.
 - The depthwise conv is split between PE (per-tap diagonal matmuls over
   shifted SBUF views of the GLU output) and DVE (scalar_tensor_tensor
   FMAs with per-partition dw taps accumulating into the same PSUM bank).
 - LayerNorm reduces over channels (= partitions) with ones-vector
   matmuls on PE; mean/rstd rows are broadcast back across partitions
   with a second 1-contraction matmul.
"""

import os
import numpy as np
import ml_dtypes

C = 1024
K = 15
LORDER = 7
B = 32
T = 512
BT = B * T           # 16384
NCORES = 8
TL = BT // NCORES    # 2048 frames per core
FD = 512             # free-dim chunk (one PSUM bank of fp32)
NCH = TL // FD       # 4 chunks per core
CT = C // 128        # 8 channel tiles
FT = 2 * C // 128    # 16 GEMM1 output-feature tiles
LN_EPS = 1e-5

# Depthwise-conv tap split between the two engines (tunable).
PE_TAPS = tuple(range(11))
DVE_TAPS = tuple(t for t in range(K) if t not in PE_TAPS)

BF16 = ml_dtypes.bfloat16

_CACHE = {}          # compiled Bass module, reused across kernel() calls
LAST_RESULT = None   # BassKernelResults of the most recent device run


def _sigmoid(v):
    return 1.0 / (1.0 + np.exp(-v))


def _glu_host(x_frames, w1, b1):
    """GLU(x @ w1.T + b1) for a small set of frames; returns (C, n)."""
    y = x_frames.astype(np.float32) @ w1.astype(np.float32).T + b1
    a, g = y[:, :C], y[:, C:]
    return (a * _sigmoid(g)).T.astype(np.float32)


def _reference_numpy(x, mask_pad, cache, truncated_context_size,
                     w1, b1, dw, db, gamma, beta, w2, b2):
    """Exact numpy translation of the reference (general fallback path)."""
    Bx, Tx, Cx = x.shape
    y = x.reshape(-1, Cx) @ w1.T + b1
    a, g = y[:, :Cx], y[:, Cx:]
    y = a * _sigmoid(g)
    flat = y.T                                            # (C, B*T)
    seq = np.concatenate([cache, flat], axis=1)
    tcs = int(truncated_context_size)
    new_cache = seq[:, tcs:tcs + LORDER].copy()
    seq = np.pad(seq, ((0, 0), (0, LORDER)))
    idx = np.arange(Tx + 2 * LORDER)[None, :] + Tx * np.arange(Bx)[:, None]
    win = np.transpose(seq[:, idx], (1, 0, 2))            # (B, C, W)
    win = np.where(mask_pad, win, 0.0)
    out = np.zeros((Bx, Cx, Tx), np.float32)
    for k in range(K):
        out += win[:, :, k:k + Tx] * dw[None, :, k:k + 1]
    out += db[None, :, None]
    z = np.transpose(out, (0, 2, 1))
    mu = z.mean(-1, keepdims=True)
    var = z.var(-1, keepdims=True)
    z = (z - mu) / np.sqrt(var + LN_EPS) * gamma + beta
    z = np.maximum(z, 0.0)
    z = z.reshape(-1, Cx) @ w2.T + b2
    z = z.reshape(Bx, Tx, Cx)
    mask_t = mask_pad[:, 0, LORDER:-LORDER][:, :, None]
    return np.where(mask_t, z, 0.0).astype(np.float32), new_cache.astype(np.float32)


def _build_nc(repeat=1):
    import concourse.bacc as bacc
    import concourse.tile as tile
    import concourse.mybir as mybir
    from contextlib import ExitStack

    f32 = mybir.dt.float32
    bf16 = mybir.dt.bfloat16
    AF = mybir.ActivationFunctionType
    OP = mybir.AluOpType

    nc = bacc.Bacc("TRN2", target_bir_lowering=False, debug=False,
                   enable_asserts=False)

    xT = nc.dram_tensor("xT", [C, TL], bf16, kind="ExternalInput").ap()
    lctx = nc.dram_tensor("lctx", [C, LORDER], bf16, kind="ExternalInput").ap()
    rctx = nc.dram_tensor("rctx", [C, LORDER], bf16, kind="ExternalInput").ap()
    w1T = nc.dram_tensor("w1T", [C, 2 * C], bf16, kind="ExternalInput").ap()
    w2T = nc.dram_tensor("w2T", [C, C], bf16, kind="ExternalInput").ap()
    b1a = nc.dram_tensor("b1a", [128, CT], f32, kind="ExternalInput").ap()
    b1g = nc.dram_tensor("b1g", [128, CT], f32, kind="ExternalInput").ap()
    dbv = nc.dram_tensor("dbv", [128, CT], f32, kind="ExternalInput").ap()
    gam = nc.dram_tensor("gam", [128, CT], f32, kind="ExternalInput").ap()
    bet = nc.dram_tensor("bet", [128, CT], f32, kind="ExternalInput").ap()
    dwc = nc.dram_tensor("dwc", [128, CT * K], f32, kind="ExternalInput").ap()
    diag = nc.dram_tensor("diag", [CT * K * 128, 128], bf16,
                          kind="ExternalInput").ap()
    z = nc.dram_tensor("z", [TL, C], f32, kind="ExternalOutput").ap()

    with tile.TileContext(nc) as tc:
        with (
            tc.tile_pool(name="persist", bufs=1) as pp,
            tc.tile_pool(name="sig", bufs=2) as sigp,
            tc.tile_pool(name="cm", bufs=10) as cmp_,
            tc.tile_pool(name="sq", bufs=3) as sqp,
            tc.tile_pool(name="rows", bufs=1) as rowp,
            tc.tile_pool(name="bc", bufs=2) as bcp,
            tc.tile_pool(name="zn", bufs=10) as znp,
            tc.tile_pool(name="o2", bufs=2) as outp,
            tc.tile_pool(name="ps_mm", bufs=6, space="PSUM") as psmm,
            tc.tile_pool(name="ps_misc", bufs=2, space="PSUM") as psm,
        ):
            # ---- persistent loads -------------------------------------
            w1sb = []
            for k in range(CT):
                t = pp.tile([128, 2 * C], bf16, tag=f"w1_{k}")
                nc.sync.dma_start(t[:], w1T[k * 128:(k + 1) * 128, :])
                w1sb.append(t)
            w2sb = []
            for k in range(CT):
                t = pp.tile([128, C], bf16, tag=f"w2_{k}")
                nc.sync.dma_start(t[:], w2T[k * 128:(k + 1) * 128, :])
                w2sb.append(t)
            xsb = []
            for k in range(CT):
                t = pp.tile([128, TL], bf16, tag=f"x_{k}")
                nc.sync.dma_start(t[:], xT[k * 128:(k + 1) * 128, :])
                xsb.append(t)
            glusb = []
            for i in range(CT):
                t = pp.tile([128, TL + 2 * LORDER], bf16, tag=f"glu_{i}")
                nc.sync.dma_start(t[:, 0:LORDER], lctx[i * 128:(i + 1) * 128, :])
                nc.sync.dma_start(t[:, TL + LORDER:TL + 2 * LORDER],
                                  rctx[i * 128:(i + 1) * 128, :])
                glusb.append(t)
            diagsb = {}
            for i in range(CT):
                for k in PE_TAPS:
                    t = pp.tile([128, 128], bf16, tag=f"diag_{i}_{k}")
                    r = (i * K + k) * 128
                    nc.sync.dma_start(t[:], diag[r:r + 128, :])
                    diagsb[(i, k)] = t

            b1a_sb = pp.tile([128, CT], f32, tag="b1a")
            nc.sync.dma_start(b1a_sb[:], b1a)
            b1g_sb = pp.tile([128, CT], f32, tag="b1g")
            nc.sync.dma_start(b1g_sb[:], b1g)
            db_sb = pp.tile([128, CT], f32, tag="dbv")
            nc.sync.dma_start(db_sb[:], dbv)
            gam_sb = pp.tile([128, CT], f32, tag="gam")
            nc.sync.dma_start(gam_sb[:], gam)
            bet_sb = pp.tile([128, CT], f32, tag="bet")
            nc.sync.dma_start(bet_sb[:], bet)
            dw_sb = pp.tile([128, CT * K], f32, tag="dwc")
            nc.sync.dma_start(dw_sb[:], dwc)
            ones_col = pp.tile([128, 1], bf16, tag="ones_col")
            nc.vector.memset(ones_col[:], 1.0)
            ones_row = pp.tile([1, 128], bf16, tag="ones_row")
            nc.vector.memset(ones_row[:], 1.0)
            eps_sb = pp.tile([1, 1], f32, tag="eps")
            nc.vector.memset(eps_sb[:], LN_EPS)

            # Optional on-device repeat loop for benchmarking.
            loop_ctx = ExitStack()
            if repeat > 1:
                loop_ctx.enter_context(
                    tc.For_i(0, repeat, 1,
                             hint_engines=(mybir.EngineType.PE,
                                           mybir.EngineType.DVE,
                                           mybir.EngineType.Activation)))

            # ---- phase 1: GEMM1 + GLU over the whole shard ------------
            # All Sigmoid activations are emitted before any sqrt-set ACT
            # op so the ACT table set switches exactly once.
            for j in range(NCH):
                for i in range(CT):        # output pair (a_i, g_i)
                    psA = psmm.tile([128, FD], f32, tag="mm")
                    psG = psmm.tile([128, FD], f32, tag="mm")
                    for k in range(CT):
                        nc.tensor.matmul(
                            psA[:],
                            w1sb[k][:, i * 128:(i + 1) * 128],
                            xsb[k][:, j * FD:(j + 1) * FD],
                            start=(k == 0), stop=(k == CT - 1))
                    for k in range(CT):
                        nc.tensor.matmul(
                            psG[:],
                            w1sb[k][:, (i + CT) * 128:(i + CT + 1) * 128],
                            xsb[k][:, j * FD:(j + 1) * FD],
                            start=(k == 0), stop=(k == CT - 1))
                    sig = sigp.tile([128, FD], f32, tag="sig")
                    nc.scalar.activation(sig[:], psG[:], AF.Sigmoid,
                                         bias=b1g_sb[:, i:i + 1], scale=1.0)
                    # glu = (a + b1a) * sigmoid(g + b1g)  -> bf16
                    nc.vector.scalar_tensor_tensor(
                        glusb[i][:, LORDER + j * FD:LORDER + (j + 1) * FD],
                        psA[:], b1a_sb[:, i:i + 1], sig[:],
                        OP.add, OP.mult)

            # ---- phase 2: depthwise conv + LN + GEMM2, per chunk ------
            for j in range(NCH):
                cms = []
                sqs = []
                for i in range(CT):
                    cv = psmm.tile([128, FD], f32, tag="mm")
                    for n, k in enumerate(PE_TAPS):
                        nc.tensor.matmul(
                            cv[:], diagsb[(i, k)][:],
                            glusb[i][:, j * FD + k:j * FD + k + FD],
                            start=(n == 0), stop=(n == len(PE_TAPS) - 1))
                    for k in DVE_TAPS:
                        nc.vector.scalar_tensor_tensor(
                            cv[:],
                            glusb[i][:, j * FD + k:j * FD + k + FD],
                            dw_sb[:, i * K + k:i * K + k + 1],
                            cv[:], OP.mult, OP.add)
                    cm = cmp_.tile([128, FD], bf16, tag="cm")
                    nc.vector.tensor_scalar(cm[:], cv[:],
                                            db_sb[:, i:i + 1], None, OP.add)
                    sq = sqp.tile([128, FD], bf16, tag="sq")
                    nc.scalar.square(sq[:], cm[:])
                    cms.append(cm)
                    sqs.append(sq)

                # LN stats over channels via ones-matmuls
                srow = psm.tile([128, FD], f32, tag="row")
                for i in range(CT):
                    nc.tensor.matmul(srow[0:1, :], ones_col[:], cms[i][:],
                                     start=(i == 0), stop=(i == CT - 1))
                mu_row = rowp.tile([1, FD], bf16, tag="mu_row")
                nc.scalar.activation(mu_row[:], srow[0:1, :], AF.Copy,
                                     scale=1.0 / C)
                musq = rowp.tile([1, FD], f32, tag="musq")
                nc.scalar.activation(musq[:], srow[0:1, :], AF.Square,
                                     scale=1.0 / C)
                qrow = psm.tile([128, FD], f32, tag="row")
                for i in range(CT):
                    nc.tensor.matmul(qrow[0:1, :], ones_col[:], sqs[i][:],
                                     start=(i == 0), stop=(i == CT - 1))
                var_row = rowp.tile([1, FD], f32, tag="var_row")
                nc.vector.scalar_tensor_tensor(var_row[:], qrow[0:1, :], 1.0 / C,
                                               musq[:], OP.mult, OP.subtract)
                std_row = rowp.tile([1, FD], f32, tag="std_row")
                nc.scalar.activation(std_row[:], var_row[:], AF.Sqrt,
                                     bias=eps_sb[:])
                rstd_f32 = rowp.tile([1, FD], f32, tag="rstd_f32")
                nc.vector.reciprocal(rstd_f32[:], std_row[:])
                rstd_row = rowp.tile([1, FD], bf16, tag="rstd_row")
                nc.scalar.copy(rstd_row[:], rstd_f32[:])

                # broadcast mu/rstd across partitions (1-contraction matmul)
                mups = psm.tile([128, FD], f32, tag="row")
                nc.tensor.matmul(mups[:], ones_row[:], mu_row[:])
                mub = bcp.tile([128, FD], bf16, tag="mub")
                nc.scalar.copy(mub[:], mups[:])
                rsps = psm.tile([128, FD], f32, tag="row")
                nc.tensor.matmul(rsps[:], ones_row[:], rstd_row[:])
                rstdb = bcp.tile([128, FD], bf16, tag="rstdb")
                nc.scalar.copy(rstdb[:], rsps[:])

                # normalize (in place on cm) + affine + relu -> bf16 operand
                zns = []
                for i in range(CT):
                    nc.vector.tensor_sub(cms[i][:], cms[i][:], mub[:])
                    nc.vector.tensor_mul(cms[i][:], cms[i][:], rstdb[:])
                    zn = znp.tile([128, FD], bf16, tag="zn")
                    nc.scalar.activation(zn[:], cms[i][:], AF.Relu,
                                         bias=bet_sb[:, i:i + 1],
                                         scale=gam_sb[:, i:i + 1])
                    zns.append(zn)

                # GEMM2: out2[t, f] = sum_c zn[c, t] * w2T[c, f]
                for tt in range(FD // 128):
                    g2a = psmm.tile([128, FD], f32, tag="mm")
                    g2b = psmm.tile([128, FD], f32, tag="mm")
                    for k in range(CT):
                        lhs = zns[k][:, tt * 128:(tt + 1) * 128]
                        nc.tensor.matmul(g2a[:], lhs, w2sb[k][:, 0:FD],
                                         start=(k == 0), stop=(k == CT - 1))
                        nc.tensor.matmul(g2b[:], lhs, w2sb[k][:, FD:2 * FD],
                                         start=(k == 0), stop=(k == CT - 1))
                    o = outp.tile([128, C], f32, tag="o2")
                    nc.scalar.copy(o[:, 0:FD], g2a[:])
                    nc.scalar.copy(o[:, FD:2 * FD], g2b[:])
                    r0 = (j * (FD // 128) + tt) * 128
                    nc.sync.dma_start(z[r0:r0 + 128, :], o[:])
            loop_ctx.close()

    nc.compile()
    return nc


def _get_nc():
    if "nc" not in _CACHE:
        _CACHE["nc"] = _build_nc()
    return _CACHE["nc"]


def kernel(x, mask_pad, cache, truncated_context_size,
           w1, b1, dw, db, gamma, beta, w2, b2):
    global LAST_RESULT
    x = np.asarray(x, np.float32)
    mask_pad = np.asarray(mask_pad)
    cache = np.asarray(cache, np.float32)
    w1 = np.asarray(w1, np.float32)
    b1 = np.asarray(b1, np.float32)
    dw = np.asarray(dw, np.float32)
    db = np.asarray(db, np.float32)
    gamma = np.asarray(gamma, np.float32)
    beta = np.asarray(beta, np.float32)
    w2 = np.asarray(w2, np.float32)
    b2 = np.asarray(b2, np.float32)
    tcs = int(truncated_context_size)

    if not bool(mask_pad.all()) or x.shape != (B, T, C):
        return _reference_numpy(x, mask_pad, cache, tcs, w1, b1, dw, db,
                                gamma, beta, w2, b2)

    x_flat = x.reshape(BT, C)

    # halo GLU values at each core boundary (host side, 7 frames each)
    def glu_at(lo, hi):
        return _glu_host(x_flat[lo:hi], w1, b1)

    lctxs, rctxs = [], []
    for i in range(NCORES):
        base = i * TL
        if i == 0:
            lctxs.append(cache)
        else:
            lctxs.append(glu_at(base - LORDER, base))
        if i == NCORES - 1:
            rctxs.append(np.zeros((C, LORDER), np.float32))
        else:
            rctxs.append(glu_at(base + TL, base + TL + LORDER))

    # new_cache = seq[:, tcs:tcs+7] where seq = [cache | glu(flat)]
    cols = np.arange(tcs, tcs + LORDER)
    new_cache = np.empty((C, LORDER), np.float32)
    need = cols >= LORDER
    if (~need).any():
        new_cache[:, ~need] = cache[:, cols[~need]]
    if need.any():
        f0, f1 = cols[need][0] - LORDER, cols[need][-1] - LORDER + 1
        new_cache[:, need] = glu_at(f0, f1)

    # host-prepped shared operands
    w1T_h = np.ascontiguousarray(w1.T).astype(BF16)
    w2T_h = np.ascontiguousarray(w2.T).astype(BF16)
    b1a_h = np.ascontiguousarray(b1[:C].reshape(CT, 128).T, np.float32)
    b1g_h = np.ascontiguousarray(b1[C:].reshape(CT, 128).T, np.float32)
    db_h = np.ascontiguousarray(db.reshape(CT, 128).T, np.float32)
    gam_h = np.ascontiguousarray(gamma.reshape(CT, 128).T, np.float32)
    bet_h = np.ascontiguousarray(beta.reshape(CT, 128).T, np.float32)
    dw_h = np.ascontiguousarray(
        dw.reshape(CT, 128, K).transpose(1, 0, 2).reshape(128, CT * K),
        np.float32)
    diag_h = np.zeros((CT * K, 128, 128), np.float32)
    for i in range(CT):
        for k in range(K):
            np.fill_diagonal(diag_h[i * K + k], dw[i * 128:(i + 1) * 128, k])
    diag_h = diag_h.reshape(CT * K * 128, 128).astype(BF16)

    in_maps = []
    for i in range(NCORES):
        base = i * TL
        in_maps.append({
            "xT": np.ascontiguousarray(x_flat[base:base + TL].T).astype(BF16),
            "lctx": lctxs[i].astype(BF16),
            "rctx": rctxs[i].astype(BF16),
            "w1T": w1T_h, "w2T": w2T_h,
            "b1a": b1a_h, "b1g": b1g_h, "dbv": db_h,
            "gam": gam_h, "bet": bet_h, "dwc": dw_h,
            "diag": diag_h,
        })

    from concourse.bass_utils import run_bass_kernel_spmd
    nc = _get_nc()
    res = run_bass_kernel_spmd(nc, in_maps, core_ids=list(range(NCORES)),
                               trace=bool(int(os.environ.get("KERNEL_TRACE", "0"))))
    LAST_RESULT = res

    z = np.concatenate([res.results[i]["z"] for i in range(NCORES)], axis=0)
    z = z.reshape(B, T, C)
    if b2.any():
        z = z + b2
    return z.astype(np.float32), new_cache
